# revision 36
# baseline (speedup 1.0000x reference)
"""MoE (cosine-routed, top-k, 2-layer GELU FFN) on 8 Trainium2 NeuronCores.

Strategy (expert-parallel, per the sharding hint):
  - Host computes the (tiny) routing: cosine scores -> softmax -> top-k ->
    renormalized gate weights. ~34 MFLOP, negligible vs the 34 GFLOP FFN.
  - Tokens are dispatched by top-k expert id: core e receives the tokens
    routed to expert e (padded to capacity C), plus expert e's W1/b1/W2/b2.
  - Each core runs the 2-layer FFN in bf16 (fp32 PSUM accumulation) and
    scales each token's output by its gate weight on-device.
  - Host scatter-adds the (<= top_k) expert contributions per token.

Device layout per core (P = 128 partitions):
  GEMM1: hT[f, t] = sum_d W1[d, f] * xT[d, t]   (W1 tiles stationary)
         -> Gelu(. + b1) on ScalarE, cast to bf16
  GEMM2: yT[d, t] = sum_f W2[f, d] * hT[f, t]   (W2 tiles stationary)
         -> (. + b2) * gate on VectorE, fp32 out

All DRAM inputs are pre-arranged on the host into the exact SBUF layout
(partition-contiguous), so every DMA moves large contiguous per-partition
segments (>= 2KB bursts; the biggest transfers are single multi-MB DMAs).
A short run of dummy matmuls on a zeroed tile warms the PE HAM clock
(1.2 -> 2.4 GHz) while the first DMAs are in flight.
"""

import numpy as np
import ml_dtypes

P = 128
D_MODEL = 1024
D_FF = 2048
N_EXPERTS = 8
N_CORES = 8
N_WARMUP_MM = 24

_BF16 = ml_dtypes.bfloat16

_cache: dict = {}
last_results = None  # BassKernelResults of the most recent run (for profiling)


def _chunks(C):
    out = []
    c0 = 0
    while c0 < C:
        cw = min(512, C - c0)
        out.append((c0, cw))
        c0 += cw
    return out


def _build(C):
    """Build + compile the SPMD FFN kernel for capacity C (multiple of 32)."""
    import concourse.mybir as mybir
    from concourse import bacc
    from concourse.tile import TileContext

    D, F = D_MODEL, D_FF
    ND, NF = D // P, F // P

    nc = bacc.Bacc("TRN2", target_bir_lowering=False, debug=False,
                   enable_partition_id=False)

    # Host-pre-arranged layouts (see kernel() for the packing):
    #   xT:  [P, ND*C]    column d*C + t       = x[token t, d*P + part]
    #   w1:  [P, NF*ND*P] column f*ND*P + d*P + j = W1[d*P + part, f*P + j]
    #   w2:  [P, NF*D]    column f*D + j       = W2[f*P + part, j]
    xT_d = nc.dram_tensor("xT", [P, ND * C], mybir.dt.bfloat16, kind="ExternalInput")
    w1_d = nc.dram_tensor("w1", [P, NF * ND * P], mybir.dt.bfloat16,
                          kind="ExternalInput")
    w2_d = nc.dram_tensor("w2", [P, NF * D], mybir.dt.bfloat16, kind="ExternalInput")
    meta_d = nc.dram_tensor("meta", [P, NF + ND + C], mybir.dt.float32,
                            kind="ExternalInput")
    out_d = nc.dram_tensor("out", [D, C], mybir.dt.float32, kind="ExternalOutput")

    ck = _chunks(C)

    with TileContext(nc) as tc:
        with (
            tc.tile_pool(name="weights", bufs=1) as wp,
            tc.tile_pool(name="acts", bufs=1) as ap,
            tc.tile_pool(name="outs", bufs=4) as op,
            tc.tile_pool(name="psum", bufs=2, space="PSUM") as pp,
        ):
            # PSUM budget: 8 banks. Chunk 0 is double-buffered when the
            # bank budget allows (<= 2 chunks); extra chunks single-buffer.
            B0 = 3 if len(ck) <= 2 else 1
            psbufs = lambda ci: B0 if ci == 0 else 1

            # --- PE warm-up: dummy matmuls on a zeroed tile, no DMA deps.
            # The junk accumulator shares chunk 0's psum slot (used first).
            dummy = ap.tile([P, P], mybir.dt.bfloat16, tag="dummy")
            nc.vector.memset(dummy[:], 0.0)
            wps = pp.tile([P, P], mybir.dt.float32, tag="ps1_0", name="warm_ps",
                          bufs=B0)
            for _ in range(N_WARMUP_MM):
                nc.tensor.matmul(wps[:], dummy[:], dummy[:], start=True, stop=True)

            xt = ap.tile([P, ND * C], mybir.dt.bfloat16, tag="xt")
            w1t = wp.tile([P, NF * ND * P], mybir.dt.bfloat16, tag="w1")
            w2t = wp.tile([P, NF * D], mybir.dt.bfloat16, tag="w2")
            mt = wp.tile([P, NF + ND + C], mybir.dt.float32, tag="meta")
            b1t = mt[:, 0:NF]
            b2t = mt[:, NF : NF + ND]
            gt = mt[:, NF + ND : NF + ND + C]
            ht = ap.tile([P, NF * C], mybir.dt.bfloat16, tag="ht")

            # --- DMAs (all fully contiguous). One logical HWDGE queue
            # drains FIFO, so issue order = arrival order; a DMA's
            # completion semaphore only fires once the WHOLE transfer is
            # in, so granularity matches consumption: x in halves, W1 per
            # f-block (one group's stationary tiles each), W2 in quarters.
            W1B = ND * P  # columns per W1 f-block
            XH = (ND // 2) * C
            nc.sync.dma_start(out=xt[:, :XH], in_=xT_d[:, :XH])
            nc.sync.dma_start(out=w1t[:, :W1B], in_=w1_d[:, :W1B])
            nc.sync.dma_start(out=xt[:, XH:], in_=xT_d[:, XH:])
            nc.sync.dma_start(out=mt[:], in_=meta_d[:])
            for f in range(1, NF):
                nc.sync.dma_start(out=w1t[:, f * W1B : (f + 1) * W1B],
                                  in_=w1_d[:, f * W1B : (f + 1) * W1B])
            NW2 = 4
            w2step = (NF // NW2) * D
            for i in range(NW2):
                nc.sync.dma_start(out=w2t[:, i * w2step : (i + 1) * w2step],
                                  in_=w2_d[:, i * w2step : (i + 1) * w2step])

            # --- GEMM1 + GELU: hT[f*P:(f+1)*P, t].
            # Chunk loop inside the d-accumulation: each W1 stationary tile
            # serves len(ck) matmuls; LDWEIGHTS hides under the wide chunk.
            for f in range(NF):
                ps = [pp.tile([P, cw], mybir.dt.float32, tag=f"ps1_{ci}",
                              name=f"ps1_{f}_{ci}", bufs=psbufs(ci))
                      for ci, (c0, cw) in enumerate(ck)]
                for d in range(ND):
                    lhs = w1t[:, f * W1B + d * P : f * W1B + (d + 1) * P]
                    for ci, (c0, cw) in enumerate(ck):
                        nc.tensor.matmul(
                            ps[ci][:],
                            lhs,
                            xt[:, d * C + c0 : d * C + c0 + cw],
                            start=(d == 0),
                            stop=(d == ND - 1),
                        )
                for ci, (c0, cw) in enumerate(ck):
                    nc.scalar.activation(
                        ht[:, f * C + c0 : f * C + c0 + cw],
                        ps[ci][:],
                        mybir.ActivationFunctionType.Gelu,
                        bias=b1t[:, f : f + 1],
                    )


            # --- GEMM2 + bias + gate: yT[do*P:(do+1)*P, t].
            for do in range(ND):
                ps2 = [pp.tile([P, cw], mybir.dt.float32, tag=f"ps2_{ci}",
                               name=f"ps2_{do}_{ci}", bufs=psbufs(ci))
                       for ci, (c0, cw) in enumerate(ck)]
                for f in range(NF):
                    lhs = w2t[:, f * D + do * P : f * D + (do + 1) * P]
                    for ci, (c0, cw) in enumerate(ck):
                        nc.tensor.matmul(
                            ps2[ci][:],
                            lhs,
                            ht[:, f * C + c0 : f * C + c0 + cw],
                            start=(f == 0),
                            stop=(f == NF - 1),
                        )
                ot = op.tile([P, C], mybir.dt.float32, tag="ot",
                             name=f"ot_{do}")
                for ci, (c0, cw) in enumerate(ck):
                    nc.vector.scalar_tensor_tensor(
                        ot[:, c0 : c0 + cw],
                        ps2[ci][:],
                        b2t[:, do : do + 1],
                        gt[:, c0 : c0 + cw],
                        op0=mybir.AluOpType.add,
                        op1=mybir.AluOpType.mult,
                    )
                    nc.sync.dma_start(
                        out=out_d[do * P : (do + 1) * P, c0 : c0 + cw],
                        in_=ot[:, c0 : c0 + cw],
                    )

    nc.compile()
    return nc


def _get_kernel(C):
    if C not in _cache:
        _cache[C] = _build(C)
    return _cache[C]


def _run_spmd(nc, in_maps):
    """run_bass_kernel_spmd, robust to a BASS_TRACE env the image can't
    serve (missing antenv.axon_hooks / artifact upload): install a best-
    effort NTFF hook shim, and on a trace-path failure fall back to an
    untraced run."""
    import os
    from concourse.bass_utils import run_bass_kernel_spmd

    try:
        import antenv.axon_hooks  # noqa: F401
    except ImportError:
        import sys
        import types
        hook = None
        try:
            from trn_agent_boot.trn_boot import _ntff_profile_via_ctypes
            hook = _ntff_profile_via_ctypes("/opt/axon/libaxon_pjrt.so")
        except Exception:
            hook = None
        mod = types.ModuleType("antenv.axon_hooks")
        mod.get_axon_ntff_profile_hook = lambda: hook
        try:
            import antenv
            antenv.axon_hooks = mod
            sys.modules["antenv.axon_hooks"] = mod
        except ImportError:
            pass

    core_ids = list(range(N_CORES))
    try:
        return run_bass_kernel_spmd(nc, in_maps, core_ids)
    except Exception:
        if os.environ.get("BASS_NEVER_TRACE") == "1":
            raise
        os.environ["BASS_NEVER_TRACE"] = "1"
        try:
            return run_bass_kernel_spmd(nc, in_maps, core_ids)
        finally:
            del os.environ["BASS_NEVER_TRACE"]


def kernel(x, anchors, temperature, W1, b1, W2, b2, top_k):

    x = np.asarray(x)
    B, S, D = x.shape
    T = B * S
    E = np.asarray(anchors).shape[0]
    k = int(np.asarray(top_k))

    xf = np.ascontiguousarray(x.reshape(T, D), dtype=np.float32)

    # ---- routing on host (part of the dispatch decision) ----
    xn = xf / np.maximum(np.linalg.norm(xf, axis=-1, keepdims=True), 1e-8)
    an = np.asarray(anchors, dtype=np.float32)
    an = an / np.maximum(np.linalg.norm(an, axis=-1, keepdims=True), 1e-8)
    scores = (xn @ an.T) * abs(float(np.asarray(temperature)))
    scores -= scores.max(axis=-1, keepdims=True)
    probs = np.exp(scores)
    probs /= probs.sum(axis=-1, keepdims=True)
    topi = np.argsort(-probs, axis=-1, kind="stable")[:, :k]  # ties -> low idx
    topv = np.take_along_axis(probs, topi, axis=-1)
    gw = topv / (topv.sum(axis=-1, keepdims=True) + 1e-6)

    rows_per_e = []
    gates_per_e = []
    for e in range(E):
        mask = topi == e
        rows = np.nonzero(mask.any(axis=-1))[0]
        g = np.where(mask[rows], gw[rows], 0.0).sum(axis=-1).astype(np.float32)
        rows_per_e.append(rows)
        gates_per_e.append(g)

    max_count = max(len(r) for r in rows_per_e)
    C = max(64, -(-max_count // 32) * 32)
    nc = _get_kernel(C)

    # ---- per-core shards, pre-arranged into SBUF layouts ----
    x_bf = xf.astype(_BF16)
    ND, NF = D_MODEL // P, D_FF // P
    in_maps = []
    for e in range(N_CORES):
        rows = rows_per_e[e]
        n = len(rows)
        xT = np.zeros((P, ND * C), dtype=_BF16)
        # [P, ND, C] view: xT[p, d, t] = x[rows[t], d*P + p]
        xv = xT.reshape(P, ND, C)
        xv[:, :, :n] = x_bf[rows].reshape(n, ND, P).transpose(2, 1, 0)
        w1 = np.asarray(W1[e], dtype=np.float32).astype(_BF16)
        w1 = np.ascontiguousarray(
            w1.reshape(ND, P, NF, P).transpose(1, 2, 0, 3).reshape(P, NF * ND * P))
        w2 = np.asarray(W2[e], dtype=np.float32).astype(_BF16)
        w2 = np.ascontiguousarray(
            w2.reshape(NF, P, D_MODEL).transpose(1, 0, 2).reshape(P, NF * D_MODEL))
        meta = np.zeros((P, NF + ND + C), dtype=np.float32)
        meta[:, :NF] = np.asarray(b1[e], dtype=np.float32).reshape(NF, P).T
        meta[:, NF : NF + ND] = np.asarray(b2[e], dtype=np.float32).reshape(ND, P).T
        meta[:, NF + ND : NF + ND + n] = gates_per_e[e][None, :]
        in_maps.append({"xT": xT, "w1": w1, "w2": w2, "meta": meta})

    res = _run_spmd(nc, in_maps)
    global last_results
    last_results = res

    # ---- combine (scatter-add the gated expert outputs) ----
    out = np.zeros((T, D_MODEL), dtype=np.float32)
    for e in range(N_CORES):
        rows = rows_per_e[e]
        n = len(rows)
        if n:
            out[rows] += res.results[e]["out"][:, :n].T
    return out.reshape(B, S, D_MODEL)


# revision 37
# speedup vs baseline: 1.0091x; 1.0091x over previous
"""MoE (cosine-routed, top-k, 2-layer GELU FFN) on 8 Trainium2 NeuronCores.

Strategy (expert-parallel, per the sharding hint):
  - Host computes the (tiny) routing: cosine scores -> softmax -> top-k ->
    renormalized gate weights. ~34 MFLOP, negligible vs the 34 GFLOP FFN.
  - Tokens are dispatched by top-k expert id: core e receives the tokens
    routed to expert e (padded to capacity C), plus expert e's W1/b1/W2/b2.
  - Each core runs the 2-layer FFN in bf16 (fp32 PSUM accumulation) and
    scales each token's output by its gate weight on-device.
  - Host scatter-adds the (<= top_k) expert contributions per token.

Device layout per core (P = 128 partitions):
  GEMM1: hT[f, t] = sum_d W1[d, f] * xT[d, t]   (W1 tiles stationary)
         -> Gelu(. + b1) on ScalarE, cast to bf16
  GEMM2: yT[d, t] = sum_f W2[f, d] * hT[f, t]   (W2 tiles stationary)
         -> (. + b2) * gate on VectorE, fp32 out

All DRAM inputs are pre-arranged on the host into the exact SBUF layout
(partition-contiguous), so every DMA moves large contiguous per-partition
segments (>= 2KB bursts; the biggest transfers are single multi-MB DMAs).
A short run of dummy matmuls on a zeroed tile warms the PE HAM clock
(1.2 -> 2.4 GHz) while the first DMAs are in flight.
"""

import numpy as np
import ml_dtypes

P = 128
D_MODEL = 1024
D_FF = 2048
N_EXPERTS = 8
N_CORES = 8
N_WARMUP_MM = 24

_BF16 = ml_dtypes.bfloat16

_cache: dict = {}
last_results = None  # BassKernelResults of the most recent run (for profiling)


def _chunks(C):
    out = []
    c0 = 0
    while c0 < C:
        cw = min(512, C - c0)
        out.append((c0, cw))
        c0 += cw
    return out


def _build(C):
    """Build + compile the SPMD FFN kernel for capacity C (multiple of 32)."""
    import concourse.mybir as mybir
    from concourse import bacc
    from concourse.tile import TileContext

    D, F = D_MODEL, D_FF
    ND, NF = D // P, F // P

    nc = bacc.Bacc("TRN2", target_bir_lowering=False, debug=False,
                   enable_partition_id=False)

    # Host-pre-arranged layouts (see kernel() for the packing):
    #   xT:  [P, ND*C]    column d*C + t       = x[token t, d*P + part]
    #   w1:  [P, NF*ND*P] column f*ND*P + d*P + j = W1[d*P + part, f*P + j]
    #   w2:  [P, NF*D]    column f*D + j       = W2[f*P + part, j]
    xT_d = nc.dram_tensor("xT", [P, ND * C], mybir.dt.bfloat16, kind="ExternalInput")
    w1_d = nc.dram_tensor("w1", [P, NF * ND * P], mybir.dt.bfloat16,
                          kind="ExternalInput")
    w2_d = nc.dram_tensor("w2", [P, NF * D], mybir.dt.bfloat16, kind="ExternalInput")
    meta_d = nc.dram_tensor("meta", [P, NF + ND + C], mybir.dt.float32,
                            kind="ExternalInput")
    out_d = nc.dram_tensor("out", [D, C], mybir.dt.float32, kind="ExternalOutput")

    ck = _chunks(C)

    with TileContext(nc) as tc:
        with (
            tc.tile_pool(name="weights", bufs=1) as wp,
            tc.tile_pool(name="acts", bufs=1) as ap,
            tc.tile_pool(name="outs", bufs=4) as op,
            tc.tile_pool(name="psum", bufs=2, space="PSUM") as pp,
        ):
            # PSUM budget: 8 banks. Chunk 0 is double-buffered when the
            # bank budget allows (<= 2 chunks); extra chunks single-buffer.
            B0 = 2 if len(ck) <= 2 else 1
            psbufs = lambda ci: B0 if ci == 0 else 1

            # --- PE warm-up: dummy matmuls on a zeroed tile, no DMA deps.
            # The junk accumulator shares chunk 0's psum slot (used first).
            dummy = ap.tile([P, P], mybir.dt.bfloat16, tag="dummy")
            nc.vector.memset(dummy[:], 0.0)
            wps = pp.tile([P, P], mybir.dt.float32, tag="ps1_0", name="warm_ps",
                          bufs=B0)
            for _ in range(N_WARMUP_MM):
                nc.tensor.matmul(wps[:], dummy[:], dummy[:], start=True, stop=True)

            xt = ap.tile([P, ND * C], mybir.dt.bfloat16, tag="xt")
            w1t = wp.tile([P, NF * ND * P], mybir.dt.bfloat16, tag="w1")
            w2t = wp.tile([P, NF * D], mybir.dt.bfloat16, tag="w2")
            mt = wp.tile([P, NF + ND + C], mybir.dt.float32, tag="meta")
            b1t = mt[:, 0:NF]
            b2t = mt[:, NF : NF + ND]
            gt = mt[:, NF + ND : NF + ND + C]
            ht = ap.tile([P, NF * C], mybir.dt.bfloat16, tag="ht")

            # --- DMAs (all fully contiguous). One logical HWDGE queue
            # drains FIFO, so issue order = arrival order; a DMA's
            # completion semaphore only fires once the WHOLE transfer is
            # in, so granularity matches consumption: x in halves, W1 per
            # f-block (one group's stationary tiles each), W2 in quarters.
            W1B = ND * P  # columns per W1 f-block
            XH = (ND // 2) * C
            nc.sync.dma_start(out=xt[:, :XH], in_=xT_d[:, :XH])
            nc.sync.dma_start(out=w1t[:, :W1B], in_=w1_d[:, :W1B])
            nc.sync.dma_start(out=xt[:, XH:], in_=xT_d[:, XH:])
            nc.sync.dma_start(out=mt[:], in_=meta_d[:])
            for f in range(1, NF):
                nc.sync.dma_start(out=w1t[:, f * W1B : (f + 1) * W1B],
                                  in_=w1_d[:, f * W1B : (f + 1) * W1B])
            NW2 = 4
            w2step = (NF // NW2) * D
            for i in range(NW2):
                nc.sync.dma_start(out=w2t[:, i * w2step : (i + 1) * w2step],
                                  in_=w2_d[:, i * w2step : (i + 1) * w2step])

            # --- GEMM1 + GELU: hT[f*P:(f+1)*P, t].
            # Chunk loop inside the d-accumulation: each W1 stationary tile
            # serves len(ck) matmuls; LDWEIGHTS hides under the wide chunk.
            for f in range(NF):
                ps = [pp.tile([P, cw], mybir.dt.float32, tag=f"ps1_{ci}",
                              name=f"ps1_{f}_{ci}", bufs=psbufs(ci))
                      for ci, (c0, cw) in enumerate(ck)]
                for d in range(ND):
                    lhs = w1t[:, f * W1B + d * P : f * W1B + (d + 1) * P]
                    for ci, (c0, cw) in enumerate(ck):
                        nc.tensor.matmul(
                            ps[ci][:],
                            lhs,
                            xt[:, d * C + c0 : d * C + c0 + cw],
                            start=(d == 0),
                            stop=(d == ND - 1),
                        )
                for ci, (c0, cw) in enumerate(ck):
                    nc.scalar.activation(
                        ht[:, f * C + c0 : f * C + c0 + cw],
                        ps[ci][:],
                        mybir.ActivationFunctionType.Gelu,
                        bias=b1t[:, f : f + 1],
                    )


            # --- GEMM2 + bias + gate: yT[do*P:(do+1)*P, t].
            for do in range(ND):
                ps2 = [pp.tile([P, cw], mybir.dt.float32, tag=f"ps2_{ci}",
                               name=f"ps2_{do}_{ci}", bufs=psbufs(ci))
                       for ci, (c0, cw) in enumerate(ck)]
                for f in range(NF):
                    lhs = w2t[:, f * D + do * P : f * D + (do + 1) * P]
                    for ci, (c0, cw) in enumerate(ck):
                        nc.tensor.matmul(
                            ps2[ci][:],
                            lhs,
                            ht[:, f * C + c0 : f * C + c0 + cw],
                            start=(f == 0),
                            stop=(f == NF - 1),
                        )
                ot = op.tile([P, C], mybir.dt.float32, tag="ot",
                             name=f"ot_{do}")
                for ci, (c0, cw) in enumerate(ck):
                    nc.vector.scalar_tensor_tensor(
                        ot[:, c0 : c0 + cw],
                        ps2[ci][:],
                        b2t[:, do : do + 1],
                        gt[:, c0 : c0 + cw],
                        op0=mybir.AluOpType.add,
                        op1=mybir.AluOpType.mult,
                    )
                    nc.sync.dma_start(
                        out=out_d[do * P : (do + 1) * P, c0 : c0 + cw],
                        in_=ot[:, c0 : c0 + cw],
                    )

    nc.compile()
    return nc


def _get_kernel(C):
    if C not in _cache:
        _cache[C] = _build(C)
    return _cache[C]


def _run_spmd(nc, in_maps):
    """run_bass_kernel_spmd, robust to a BASS_TRACE env the image can't
    serve (missing antenv.axon_hooks / artifact upload): install a best-
    effort NTFF hook shim, and on a trace-path failure fall back to an
    untraced run."""
    import os
    from concourse.bass_utils import run_bass_kernel_spmd

    try:
        import antenv.axon_hooks  # noqa: F401
    except ImportError:
        import sys
        import types
        hook = None
        try:
            from trn_agent_boot.trn_boot import _ntff_profile_via_ctypes
            hook = _ntff_profile_via_ctypes("/opt/axon/libaxon_pjrt.so")
        except Exception:
            hook = None
        mod = types.ModuleType("antenv.axon_hooks")
        mod.get_axon_ntff_profile_hook = lambda: hook
        try:
            import antenv
            antenv.axon_hooks = mod
            sys.modules["antenv.axon_hooks"] = mod
        except ImportError:
            pass

    core_ids = list(range(N_CORES))
    try:
        return run_bass_kernel_spmd(nc, in_maps, core_ids)
    except Exception:
        if os.environ.get("BASS_NEVER_TRACE") == "1":
            raise
        os.environ["BASS_NEVER_TRACE"] = "1"
        try:
            return run_bass_kernel_spmd(nc, in_maps, core_ids)
        finally:
            del os.environ["BASS_NEVER_TRACE"]


def kernel(x, anchors, temperature, W1, b1, W2, b2, top_k):

    x = np.asarray(x)
    B, S, D = x.shape
    T = B * S
    E = np.asarray(anchors).shape[0]
    k = int(np.asarray(top_k))

    xf = np.ascontiguousarray(x.reshape(T, D), dtype=np.float32)

    # ---- routing on host (part of the dispatch decision) ----
    xn = xf / np.maximum(np.linalg.norm(xf, axis=-1, keepdims=True), 1e-8)
    an = np.asarray(anchors, dtype=np.float32)
    an = an / np.maximum(np.linalg.norm(an, axis=-1, keepdims=True), 1e-8)
    scores = (xn @ an.T) * abs(float(np.asarray(temperature)))
    scores -= scores.max(axis=-1, keepdims=True)
    probs = np.exp(scores)
    probs /= probs.sum(axis=-1, keepdims=True)
    topi = np.argsort(-probs, axis=-1, kind="stable")[:, :k]  # ties -> low idx
    topv = np.take_along_axis(probs, topi, axis=-1)
    gw = topv / (topv.sum(axis=-1, keepdims=True) + 1e-6)

    rows_per_e = []
    gates_per_e = []
    for e in range(E):
        mask = topi == e
        rows = np.nonzero(mask.any(axis=-1))[0]
        g = np.where(mask[rows], gw[rows], 0.0).sum(axis=-1).astype(np.float32)
        rows_per_e.append(rows)
        gates_per_e.append(g)

    max_count = max(len(r) for r in rows_per_e)
    C = max(64, -(-max_count // 32) * 32)
    nc = _get_kernel(C)

    # ---- per-core shards, pre-arranged into SBUF layouts ----
    x_bf = xf.astype(_BF16)
    ND, NF = D_MODEL // P, D_FF // P
    in_maps = []
    for e in range(N_CORES):
        rows = rows_per_e[e]
        n = len(rows)
        xT = np.zeros((P, ND * C), dtype=_BF16)
        # [P, ND, C] view: xT[p, d, t] = x[rows[t], d*P + p]
        xv = xT.reshape(P, ND, C)
        xv[:, :, :n] = x_bf[rows].reshape(n, ND, P).transpose(2, 1, 0)
        w1 = np.asarray(W1[e], dtype=np.float32).astype(_BF16)
        w1 = np.ascontiguousarray(
            w1.reshape(ND, P, NF, P).transpose(1, 2, 0, 3).reshape(P, NF * ND * P))
        w2 = np.asarray(W2[e], dtype=np.float32).astype(_BF16)
        w2 = np.ascontiguousarray(
            w2.reshape(NF, P, D_MODEL).transpose(1, 0, 2).reshape(P, NF * D_MODEL))
        meta = np.zeros((P, NF + ND + C), dtype=np.float32)
        meta[:, :NF] = np.asarray(b1[e], dtype=np.float32).reshape(NF, P).T
        meta[:, NF : NF + ND] = np.asarray(b2[e], dtype=np.float32).reshape(ND, P).T
        meta[:, NF + ND : NF + ND + n] = gates_per_e[e][None, :]
        in_maps.append({"xT": xT, "w1": w1, "w2": w2, "meta": meta})

    res = _run_spmd(nc, in_maps)
    global last_results
    last_results = res

    # ---- combine (scatter-add the gated expert outputs) ----
    out = np.zeros((T, D_MODEL), dtype=np.float32)
    for e in range(N_CORES):
        rows = rows_per_e[e]
        n = len(rows)
        if n:
            out[rows] += res.results[e]["out"][:, :n].T
    return out.reshape(B, S, D_MODEL)
